# revision 28
# baseline (speedup 1.0000x reference)
"""nn_ConvblockWithTarget on 8 axon-tunneled TRN2 NeuronCores.

The tunnel (IFRT gRPC proxy) dominates: ~83 ms protocol round trip per
dispatch plus ~50-65 MB/s each way, strictly up -> RTT -> down. Design
is therefore wire-minimal; on-device compute is fully hidden behind the
fixed launch overhead (measured: running half the chunk loop changes
wall time by ~0).

- Data-parallel: 1 batch per core; no collectives.
- Input: x quantized host-side to 8-bit sinh-companded codes (accuracy
  of 9-bit uniform). Device decodes with two Exp passes; the compander
  scale cancels in LayerNorm and is folded into the baked weights.
- Output: 8-bit asinh-companded codes written in the kernel's native
  c-major fold (zero on-device transposes); host unfolds + sinh-decodes.
- Consts (block-diagonal lhsT planes) are baked into the NEFF via
  inline_tensor: zero wire bytes per call; _CACHE keys on their hash.
- Dispatch binds bass_exec directly with inputs only (no donated
  zero-output upload), cached jit, one retry for transient tunnel faults.

Wire: 8.39 MB up + 4.19 MB down; ~295 ms/call vs 616 ms baseline.
Accuracy: rel err 1.311e-2 (gate 2e-2), matching the numpy error model.
"""
import sys

sys.path.insert(0, "/opt/trn_rl_repo")

import numpy as np

# Problem constants (hardcoded per contract)
B, L, C, K = 8, 16384, 64, 7
T = (L - 2 * K) // 2 + 1  # 8186
HALF = 4096               # t's per half (half-1 ragged: 8186-4096=4090, padded)
TC = 512                  # t-chunk
NCH = HALF // TC          # 8 chunks
WX = 4104                 # column width of folded x tensors (HALF + 8 pad)
LN_EPS = 1e-6

# 8-bit sinh-companded x quantization:
#   u = round(asinh(x/CX)/UX*128 + 128) in [0,255]; u=128 -> x=0 exactly
# decode on device: XA = exp(v) - exp(-v) = 2*sinh(v) = (2/CX)*x_hat, where
# v = UX*(u-128)/128. The uniform (2/CX) scale cancels in LayerNorm; the
# dynamic-conv weight planes are pre-scaled by CX/2 so tanh sees true values.
CX = 0.75
RX = 5.3
UX = float(np.arcsinh(RX / CX))
SU = UX / 128.0
# 8-bit sinh-companded out quantization: v = round(asinh(out/CO)/UO*128+128)
CO = 1.0
RO = 7.0
UO = float(np.arcsinh(RO / CO))

# packed consts layout (f16, [128, NCONST], baked into the NEFF). Matmul
# lhsT blocks are BLOCK-DIAGONAL [128, 128] (top-left block for the x
# first-half channels on partitions 0:64, bottom-right for the second
# half) so one full-array matmul contracts both halves at once.
O_WT = 0            # 896 cols: 7 x (128,128) block-diag dyn-conv planes
O_CK = 896          # 128 cols: block-diag 1x1 conv kernel (lhsT)
O_ON = 1024         # 128 cols: block-diag ones/64 (LN stats lhsT)
O_SC = 1152         # ln_scale col
O_SB = 1153         # ln_bias col
O_CB = 1154         # conv_bias col (col 1155 spare pad)
NCONST = 1156

# xi (u8) column layout: [Qe | Qo]; consts ship inside the NEFF (Const
# DRAM tensor via inline_tensor, loaded to HBM once at model load) so they
# cost zero wire bytes per call.
HW2 = 2 * WX                # 8208 (Q planes)
NIN = HW2                   # 8208

_CACHE = {}


def _build(prelu_slope: float, need_lnsb: bool, need_cb: bool,
           cn_full: "np.ndarray"):
    import concourse.bacc as bacc
    import concourse.mybir as mybir
    import concourse.tile as tile

    f32 = mybir.dt.float32
    f16 = mybir.dt.float16
    u8 = mybir.dt.uint8
    u16 = mybir.dt.uint16
    AF = mybir.ActivationFunctionType
    OP = mybir.AluOpType

    nc = bacc.Bacc("TRN2", target_bir_lowering=False, debug=False, num_devices=8)

    # ---- DRAM parameters (per-core shard data) ----
    dXI = nc.declare_dram_parameter("xi", [128, NIN], u8, isOutput=False)
    # c-major output: rows 0:64 = channels for t in [0, HALF), rows 64:128
    # = channels for t in [HALF, 2*HALF) (tail cols past T-HALF are pad).
    # No on-device transpose needed; the host unfolds during decode.
    dOUT = nc.declare_dram_parameter("out", [128, HALF], u8, isOutput=True)
    # consts baked into the NEFF ([128, NCONST] f16, both halves duplicated)
    dCN = nc.inline_tensor(cn_full, name="cn")

    from contextlib import ExitStack

    with ExitStack() as es:
        tc = es.enter_context(tile.TileContext(nc))
        cp = es.enter_context(tc.tile_pool(name="const", bufs=1))
        dh = es.enter_context(tc.tile_pool(name="dech", bufs=2))
        dn = es.enter_context(tc.tile_pool(name="decn", bufs=2))
        dv = es.enter_context(tc.tile_pool(name="decv", bufs=2))
        gp = es.enter_context(tc.tile_pool(name="gps", bufs=2, space="PSUM"))
        yp = es.enter_context(tc.tile_pool(name="yps", bufs=2))
        zp = es.enter_context(tc.tile_pool(name="zps", bufs=1, space="PSUM"))
        sp = es.enter_context(tc.tile_pool(name="sps", bufs=1, space="PSUM"))
        hp = es.enter_context(tc.tile_pool(name="hsb", bufs=10))
        pp = es.enter_context(tc.tile_pool(name="prod", bufs=16))
        ypool = es.enter_context(tc.tile_pool(name="ysb", bufs=3))
        st1 = es.enter_context(tc.tile_pool(name="st1", bufs=3))
        st2 = es.enter_context(tc.tile_pool(name="st2", bufs=3))
        st3 = es.enter_context(tc.tile_pool(name="st3", bufs=3))
        st4 = es.enter_context(tc.tile_pool(name="st4", bufs=3))
        st5 = es.enter_context(tc.tile_pool(name="st5", bufs=3))
        ynp = es.enter_context(tc.tile_pool(name="ynp", bufs=3))
        pzp = es.enter_context(tc.tile_pool(name="pzp", bufs=3))
        ofp = es.enter_context(tc.tile_pool(name="ofp", bufs=3))
        vfp = es.enter_context(tc.tile_pool(name="vfp", bufs=3))
        vup = es.enter_context(tc.tile_pool(name="vup", bufs=3))
        e1p = es.enter_context(tc.tile_pool(name="e1p", bufs=3))
        e2p = es.enter_context(tc.tile_pool(name="e2p", bufs=3))
        ohp = es.enter_context(tc.tile_pool(name="ohp", bufs=3))
        oup = es.enter_context(tc.tile_pool(name="oup", bufs=4))
        if True:
            # ---- load packed input ----
            XIN = cp.tile([128, NIN], u8)
            nc.sync.dma_start(XIN[:], dXI[:])
            EPS = cp.tile([128, 1], f32)
            nc.vector.memset(EPS[:], LN_EPS)
            ONE1 = cp.tile([128, 1], f32)
            nc.vector.memset(ONE1[:], 1.0)
            BXN = cp.tile([128, 1], f32)
            nc.vector.memset(BXN[:], -float(UX))
            BXP = cp.tile([128, 1], f32)
            nc.vector.memset(BXP[:], float(UX))

            # consts: DMA the baked Const DRAM block straight to SBUF
            CN = cp.tile([128, NCONST], f16)
            nc.sync.dma_start(CN[:], dCN[:])
            WT = CN[:, O_WT:O_WT + 896]
            CKt = CN[:, O_CK:O_CK + 128]
            ON = CN[:, O_ON:O_ON + 128]
            SCB = cp.tile([128, 3], f32)
            if need_lnsb or need_cb:
                nc.scalar.copy(SCB[:], CN[:, O_SC:O_SC + 3])

            # ---- decode 8-bit x -> XA f16 [128, 2*WX] = [xe | xo] ----
            # XA = exp(SU*u - UX) - exp(UX - SU*u) = (2/CX) * x_hat
            # Exp reads the u8 codes directly; e/o chunks interleaved so the
            # first compute chunk's inputs decode first.
            XA = cp.tile([128, 2 * WX], f16)
            CW = 1026
            for ci in range(WX // CW):
                for c0 in (CW * ci, WX + CW * ci):
                    E1 = dn.tile([128, CW], f32)
                    nc.scalar.activation(E1[:], XIN[:, c0:c0 + CW], AF.Exp,
                                         scale=float(SU), bias=BXN[:, 0:1])
                    E2 = dv.tile([128, CW], f32)
                    nc.scalar.activation(E2[:], XIN[:, c0:c0 + CW], AF.Exp,
                                         scale=-float(SU), bias=BXP[:, 0:1])
                    nc.vector.tensor_sub(XA[:, c0:c0 + CW], E1[:], E2[:])
            XE = XA[:, 0:WX]
            XO = XA[:, WX:2 * WX]

            for i in range(NCH):
                t0 = TC * i
                # ---- G matmuls + tanh: 7 m-planes, each (Ge|Go) (128,1024) ----
                hts = []
                for m in range(K):
                    g = gp.tile([128, 1024], f32)
                    for ci, src_ in ((0, XE), (512, XO)):
                        nc.tensor.matmul(
                            g[:, ci:ci + TC],
                            lhsT=WT[:, 128 * m:128 * m + 128],
                            rhs=src_[:, t0 + 6:t0 + 6 + TC],
                            start=True, stop=True,
                        )
                    ht = hp.tile([128, 1024], f16)
                    nc.scalar.activation(ht[:], g[:], AF.Tanh)
                    hts.append(ht)

                # ---- gating products (14 planes) ----
                prods = []
                for m in range(K):
                    for ci, xa in ((0, XE), (512, XO)):
                        pr = pp.tile([128, TC], f16)
                        nc.vector.tensor_mul(pr[:], xa[:, t0 + m:t0 + m + TC],
                                             hts[m][:, ci:ci + TC])
                        prods.append(pr)

                # ---- accumulate 14 products + skip via vector adds ----
                ya = yp.tile([128, TC], f32)
                nc.vector.tensor_add(ya[:], prods[0][:], prods[1][:])
                for pr in prods[2:]:
                    nc.vector.tensor_add(ya[:], ya[:], pr[:])
                nc.vector.tensor_add(ya[:], ya[:],
                                     XE[:, t0 + 6:t0 + 6 + TC])

                # ---- drain y, square ----
                ysb = ypool.tile([128, TC], f16)
                nc.scalar.copy(ysb[:], ya[:])
                ysq = pp.tile([128, TC], f16)
                nc.vector.tensor_mul(ysq[:], ysb[:], ysb[:])

                # ---- LN stats: mean & mean-of-squares via ones-matmul ----
                st = sp.tile([128, 1024], f32)
                nc.tensor.matmul(st[:, 0:TC], lhsT=ON,
                                 rhs=ysb[:], start=True, stop=True)
                nc.tensor.matmul(st[:, 512:512 + TC], lhsT=ON,
                                 rhs=ysq[:], start=True, stop=True)
                mu = st[:, 0:TC]
                m2 = st[:, 512:512 + TC]

                musq = st1.tile([128, TC], f32)
                nc.scalar.activation(musq[:], mu, AF.Square)
                var = st2.tile([128, TC], f32)
                nc.vector.tensor_sub(var[:], m2, musq[:])
                std = st3.tile([128, TC], f32)
                nc.scalar.activation(std[:], var[:], AF.Sqrt, bias=EPS[:, 0:1])
                rstd = st4.tile([128, TC], f32)
                scr = st5.tile([128, TC], f32)
                nc.vector.reciprocal_approx_accurate(rstd[:], std[:], scr[:])

                # ---- yn = (y - mu) * rstd  (* s + b) ----
                yc = st1.tile([128, TC], f32)
                nc.vector.tensor_sub(yc[:], ysb[:], mu)
                yn = ynp.tile([128, TC], f16)
                nc.vector.tensor_mul(yn[:], yc[:], rstd[:])
                if need_lnsb:
                    yn2 = ynp.tile([128, TC], f16)
                    nc.vector.tensor_scalar(yn2[:], yn[:], SCB[:, 0:1],
                                            SCB[:, 1:2],
                                            op0=OP.mult, op1=OP.add)
                    yn = yn2

                # ---- 1x1 conv ----
                z = zp.tile([128, TC], f32)
                nc.tensor.matmul(z[:], lhsT=CKt, rhs=yn[:],
                                 start=True, stop=True)
                if need_cb:
                    z2 = st2.tile([128, TC], f32)
                    nc.vector.tensor_scalar(z2[:], z[:], SCB[:, 2:3],
                                            None, op0=OP.add)
                    zsrc = z2
                else:
                    zsrc = z
                # prelu: max(z, slope*z)
                pz = pzp.tile([128, TC], f16)
                nc.scalar.activation(pz[:], zsrc[:], AF.Prelu,
                                     alpha=float(prelu_slope))

                # ---- of = yn + pz in c-layout; asinh-encode to u16 codes ----
                of = ofp.tile([128, TC], f16)
                nc.vector.tensor_add(of[:], yn[:], pz[:])
                # v = clamp(round(asinh(of/CO)*128/UO + 128)) (CO == 1)
                S1 = vfp.tile([128, TC], f32)
                nc.scalar.activation(S1[:], of[:], AF.Square)
                S2 = vup.tile([128, TC], f32)
                nc.scalar.activation(S2[:], S1[:], AF.Sqrt,
                                     bias=ONE1[:, 0:1])
                S3 = e1p.tile([128, TC], f32)
                nc.vector.tensor_add(S3[:], S2[:], of[:])
                A = e2p.tile([128, TC], f32)
                nc.scalar.activation(A[:], S3[:], AF.Ln)
                V = ohp.tile([128, TC], f32)
                nc.scalar.activation(V[:], A[:], AF.Copy,
                                     scale=float(128.0 / UO), bias=128.0)
                nc.vector.tensor_scalar(V[:], V[:], 0.0, 255.0,
                                        op0=OP.max, op1=OP.min)
                OU = oup.tile([128, TC], u8)
                nc.scalar.copy(OU[:], V[:])
                nc.sync.dma_start(dOUT[:, t0:t0 + TC], OU[:])

    nc.compile()
    return nc


def _make_runner(nc):
    """Cached 8-core dispatch without donated zero output buffers.

    run_bass_kernel_spmd (axon path) uploads a zero-filled copy of every
    output tensor per call purely to seed donated buffers; this kernel
    writes every output byte, so those H2D bytes are pure overhead. This
    runner binds the same bass_exec custom call with inputs only.
    """
    import jax
    from jax.sharding import Mesh, PartitionSpec
    from jax.experimental.shard_map import shard_map
    import concourse.mybir as mybir
    from concourse.bass2jax import (install_neuronx_cc_hook, _bass_exec_p,
                                    partition_id_tensor)

    install_neuronx_cc_hook()
    partition_name = (nc.partition_id_tensor.name
                      if nc.partition_id_tensor else None)
    in_names, out_names, out_avals = [], [], []
    for alloc in nc.m.functions[0].allocations:
        if not isinstance(alloc, mybir.MemoryLocationSet):
            continue
        name = alloc.memorylocations[0].name
        if alloc.kind == "ExternalInput":
            if name != partition_name:
                in_names.append(name)
        elif alloc.kind == "ExternalOutput":
            out_names.append(name)
            out_avals.append(jax.core.ShapedArray(
                tuple(alloc.tensor_shape), mybir.dt.np(alloc.dtype)))
    bind_in_names = tuple(in_names) + ((partition_name,)
                                       if partition_name else ())

    def _body(*args):
        operands = list(args)
        if partition_name is not None:
            operands.append(partition_id_tensor())
        return tuple(_bass_exec_p.bind(
            *operands, out_avals=tuple(out_avals),
            in_names=bind_in_names, out_names=tuple(out_names),
            lowering_input_output_aliases=(), sim_require_finite=True,
            sim_require_nnan=True, nc=nc))

    devices = jax.devices()[:8]
    mesh = Mesh(np.asarray(devices), ("core",))
    sharded = jax.jit(shard_map(
        _body, mesh=mesh,
        in_specs=(PartitionSpec("core"),) * len(in_names),
        out_specs=(PartitionSpec("core"),) * len(out_names),
        check_rep=False), keep_unused=True)
    out_shape = out_avals[0].shape

    def run(xi_concat: np.ndarray) -> np.ndarray:
        try:
            outs = sharded(xi_concat)
            return np.asarray(outs[0]).reshape(8, *out_shape)
        except Exception:
            # the axon tunnel occasionally throws transient INTERNAL
            # errors; the dispatch is idempotent, so retry once
            outs = sharded(xi_concat)
            return np.asarray(outs[0]).reshape(8, *out_shape)

    return run


_RUNNERS = {}


def run_device(nc, xi_concat: np.ndarray) -> np.ndarray:
    """One full device round trip: H2D (8 shards), execute, D2H."""
    key = id(nc)
    if key not in _RUNNERS:
        _RUNNERS[key] = _make_runner(nc)
    return _RUNNERS[key](xi_concat)


def _prep_consts(weights, ln_scale, ln_bias, conv_kernel, conv_bias):
    """[128, NCONST] f16 const block with block-diagonal lhsT matrices."""
    U = np.zeros((128, NCONST), np.float16)

    def bdiag(dst_col, blk64):  # place [64,64] on both diagonal blocks
        U[0:64, dst_col:dst_col + 64] = blk64
        U[64:128, dst_col + 64:dst_col + 128] = blk64

    for m in range(K):
        wmT = np.asarray(weights[:, :, m]).T.astype(np.float32)  # (c_in, d)
        bdiag(O_WT + 128 * m, (wmT * (CX / 2)).astype(np.float16))
    bdiag(O_CK, np.asarray(conv_kernel).astype(np.float16))
    bdiag(O_ON, np.full((64, 64), 1.0 / 64, np.float16))
    U[0:64, O_SC] = np.asarray(ln_scale, np.float16)
    U[64:128, O_SC] = np.asarray(ln_scale, np.float16)
    U[0:64, O_SB] = np.asarray(ln_bias, np.float16)
    U[64:128, O_SB] = np.asarray(ln_bias, np.float16)
    U[0:64, O_CB] = np.asarray(conv_bias, np.float16)
    U[64:128, O_CB] = np.asarray(conv_bias, np.float16)
    return np.ascontiguousarray(U)


def _prep_inputs(x, *unused_const_args):
    """Host-side prep: one packed u8 tensor, concatenated across cores."""
    xf = np.asarray(x, dtype=np.float32)
    # 8-bit sinh-companded quantize even/odd planes: (B, 64, 8192) each
    q = np.clip(np.rint(np.arcsinh(xf / CX) * (128.0 / UX) + 128.0),
                0, 255).astype(np.uint8)
    qe = q[:, 0::2, :].transpose(0, 2, 1)
    qo = q[:, 1::2, :].transpose(0, 2, 1)

    def fold(a):  # (B, 64, 8192) u8 -> (B, 128, WX)
        out = np.full((B, 128, WX), 128, np.uint8)
        out[:, 0:64, :] = a[:, :, 0:WX]
        out[:, 64:128, 0:8192 - HALF] = a[:, :, HALF:]
        return out

    XI = np.empty((B, 128, NIN), np.uint8)
    XI[:, :, 0:WX] = fold(qe)
    XI[:, :, WX:HW2] = fold(qo)

    return np.ascontiguousarray(XI.reshape(B * 128, NIN))


def _decode_out(raw):
    """(B, 128, HALF) u8 c-major folded codes -> (B, T, 64) f32"""
    v = np.empty((B, T, 64), np.float32)
    v[:, :HALF, :] = raw[:, 0:64, :].transpose(0, 2, 1)
    v[:, HALF:, :] = raw[:, 64:128, 0:T - HALF].transpose(0, 2, 1)
    return (CO * np.sinh((v - 128.0) * (UO / 128.0))).astype(np.float32)


def kernel(x, weights, ln_scale, ln_bias, conv_kernel, conv_bias, prelu_slope):
    slope = float(np.asarray(prelu_slope))
    need_lnsb = not (np.allclose(np.asarray(ln_scale), 1.0)
                     and np.allclose(np.asarray(ln_bias), 0.0))
    need_cb = not np.allclose(np.asarray(conv_bias), 0.0)

    cn_full = _prep_consts(weights, ln_scale, ln_bias, conv_kernel, conv_bias)
    key = (slope, need_lnsb, need_cb, cn_full.tobytes())
    if key not in _CACHE:
        _CACHE[key] = _build(slope, need_lnsb, need_cb, cn_full)
    nc = _CACHE[key]

    xi = _prep_inputs(x)
    raw = run_device(nc, xi)
    return _decode_out(raw)
